# revision 16
# baseline (speedup 1.0000x reference)
"""Trainium2 Bass kernel for nn_AttentionalGNN (8-core SPMD, sequence-sharded) v2.

Strategy (vs v1 baseline):
  - N=1024 node axis sharded over 8 cores (CH=128 queries each); k/v computed
    locally per (branch, source) pair and shared via AllGather (split into two
    collectives per layer: br0 first so stage B can start while the rest fly).
  - All weights of one (layer, branch) packed host-side into single DRAM blobs
    -> one DMA each (was ~14).  k/v chunks packed [128, 512] per pair -> one
    write DMA + one 1MB gather-read DMA per prop (was ~10 small ones).
  - Dead code pruned: layer-3 d2/d3 updates, layer-4 d2/d3 props + LNs are
    never observable in the output (layer 5 only needs d0/d1).
  - Single ACT table set (natural_log_exp_and_others): LN rsqrt is computed as
    exp(-0.5*ln(var+eps)) instead of Sqrt, so the exp table never reloads.
  - Scores via two concurrent K=64 row-tiles (heads packed in partitions),
    exp over [128,1024] 2-bank PSUM tiles (amortizes ACT per-op overhead).
  - d state kept in bf16; residual+LN fused ops; per-prop DVE op count cut.
  - Final layer-5 score collapses to two projections + tiny AllGather
    (out[m] = (1/32)(Wq5 @ mean_n d1)^T (Wk5 @ d0)[:, m]).
"""
import numpy as np

import concourse.bass as bass
import concourse.bacc as bacc
import concourse.mybir as mybir
import concourse.tile as tile
from concourse.bass_utils import run_bass_kernel_spmd

D, N, H, DH = 256, 1024, 4, 64
NC = 8
CH = N // NC  # 128 positions per core
F32 = mybir.dt.float32
BF16 = mybir.dt.bfloat16
AF = mybir.ActivationFunctionType

# props per layer (br, xi, si), dead ones pruned (output only needs d0/d1
# after layer 4; layer-5 'cross' collapses into the epilogue).
LAYER_PROPS = [
    [(0, 0, 0), (0, 1, 1), (1, 2, 2), (2, 3, 3)],
    [(0, 0, 1), (0, 1, 0), (1, 2, 1), (1, 1, 2), (2, 0, 3), (2, 3, 0)],
    [(0, 0, 0), (0, 1, 1), (1, 2, 2), (2, 3, 3)],
    [(0, 0, 1), (0, 1, 0), (1, 1, 2), (2, 0, 3)],
    [(0, 0, 0), (0, 1, 1)],
]
LAYER_LNS = [[0, 1, 2, 3], [0, 1, 2, 3], [0, 1, 2, 3], [0, 1], [0, 1]]
PERM = np.array([4 * (r % 64) + r // 64 for r in range(256)])

_cache = {}
import os as _os
EXP_SPLIT = _os.environ.get("K_EXP_SPLIT", "0") == "1"
SC_ROWSPLIT = _os.environ.get("K_SC_ROWSPLIT", "0") == "1"

# The act-table picker binds each function to the first table set containing
# it, so Exp->exp_and_others while Ln->natural_log_exp_and_others, and every
# LayerNorm causes two ~2.7us table reloads between softmax exps.  Removing
# Exp/Copy/Identity from the sets that lack Ln (dict order and hence set ids
# are preserved; natural_log_exp_and_others genuinely contains all of them)
# makes every activation resolve to that one set -> a single table load.
import functools as _functools
import concourse.hw_specs as _hw_specs
import concourse.bacc as _bacc_mod
_orig_gat = _hw_specs.get_activation_tables


@_functools.cache
def _gat_one_set(arch):
    tabs = _orig_gat(arch)
    keep = "natural_log_exp_and_others"
    if keep not in tabs:
        return tabs
    shared = tabs[keep]
    out = {}
    for name, fns in tabs.items():
        out[name] = fns if name == keep else (set(fns) - shared)
    return out


_hw_specs.get_activation_tables = _gat_one_set
_bacc_mod.get_activation_tables = _gat_one_set


def build_kernel(trace_scopes=False, n_layers=5, stages="ABC", blvl=9, reps=1,
                 zb=True, ln_triv=True):
    nc = bacc.Bacc("TRN2", target_bir_lowering=False, debug=False, num_devices=NC)

    # ---- I/O ----
    xcb = nc.dram_tensor("xcb", [128, 4, 2, CH], BF16, kind="ExternalInput")
    wpackA = nc.dram_tensor("wpackA", [5, 3, 128, 1024], BF16, kind="ExternalInput")
    wpackB = nc.dram_tensor("wpackB", [5, 3, 128, 4096], BF16, kind="ExternalInput")
    w5T = nc.dram_tensor("w5T", [2, 256, 256], BF16, kind="ExternalInput")  # [qT,kT]
    pb5 = nc.dram_tensor("pb5", [2, 256], F32, kind="ExternalInput")
    if not zb:
        pbq = nc.dram_tensor("pbq", [5, 3, 256], F32, kind="ExternalInput")
        pbk = nc.dram_tensor("pbk", [5, 3, 256], F32, kind="ExternalInput")
        pbv = nc.dram_tensor("pbv", [5, 3, 256], F32, kind="ExternalInput")
        mbb = nc.dram_tensor("mbb", [5, 3, 256], F32, kind="ExternalInput")
        b1b = nc.dram_tensor("b1b", [5, 3, 512], F32, kind="ExternalInput")
        b2b = nc.dram_tensor("b2b", [5, 3, 256], F32, kind="ExternalInput")
    if not ln_triv:
        lng = nc.dram_tensor("lng", [5, 4, 256], F32, kind="ExternalInput")
        lnb = nc.dram_tensor("lnb", [5, 4, 256], F32, kind="ExternalInput")
    out_d = nc.dram_tensor("out", [1, CH], F32, kind="ExternalOutput")

    # per-(layer, group) collective buffers
    def grp_split(props):
        g0 = [j for j, p in enumerate(props) if p[0] == 0]
        g1 = [j for j, p in enumerate(props) if p[0] != 0]
        return g0, g1

    agin_t, agout_t = [], []
    for i in range(5):
        g0, g1 = grp_split(LAYER_PROPS[i])
        ins, outs = [], []
        for g, idxs in enumerate([g0, g1]):
            if idxs:
                ins.append(nc.dram_tensor(f"agin{i}_{g}", [len(idxs), 128, 512], BF16))
                outs.append(nc.dram_tensor(f"agout{i}_{g}", [NC, len(idxs), 128, 512],
                                           BF16, addr_space="Shared"))
            else:
                ins.append(None)
                outs.append(None)
        agin_t.append(ins)
        agout_t.append(outs)
    ag2in = nc.dram_tensor("ag2in", [2, 128, 1], F32)
    ag2out = nc.dram_tensor("ag2out", [NC, 2, 128, 1], F32, addr_space="Shared")

    # persistent SBUF state (bf16 d-state; f32 delta accumulator)
    dstb = nc.alloc_sbuf_tensor("dstateb", [128, 4, 2, CH], BF16)
    dlt = nc.alloc_sbuf_tensor("delta", [128, 4, 2 * CH], F32)

    rg = [list(range(NC))]

    from contextlib import ExitStack
    with ExitStack() as es:
        tc = es.enter_context(tile.TileContext(nc))
        cpool = es.enter_context(tc.tile_pool(name="const", bufs=1))
        wbp = es.enter_context(tc.tile_pool(name="wb", bufs=3))
        wap = es.enter_context(tc.tile_pool(name="wa", bufs=3))
        kvwp = es.enter_context(tc.tile_pool(name="kvw", bufs=4))
        kvgp = es.enter_context(tc.tile_pool(name="kvg", bufs=2))
        qhp = es.enter_context(tc.tile_pool(name="qh", bufs=6))
        ep = es.enter_context(tc.tile_pool(name="exps", bufs=9))
        zap = es.enter_context(tc.tile_pool(name="zacc", bufs=2))
        sp_ = es.enter_context(tc.tile_pool(name="small", bufs=4))
        abp = es.enter_context(tc.tile_pool(name="bcsb", bufs=2))
        atp = es.enter_context(tc.tile_pool(name="attn", bufs=2))
        msp = es.enter_context(tc.tile_pool(name="msg", bufs=2))
        h1p = es.enter_context(tc.tile_pool(name="h1", bufs=4))
        lnp = es.enter_context(tc.tile_pool(name="ln", bufs=2))
        tmpp = es.enter_context(tc.tile_pool(name="tmp", bufs=2))
        bp = es.enter_context(tc.tile_pool(name="bias", bufs=3))
        scp = es.enter_context(tc.tile_pool(name="sc", bufs=2, space="PSUM"))
        avp = es.enter_context(tc.tile_pool(name="av", bufs=1, space="PSUM"))
        psp = es.enter_context(tc.tile_pool(name="ps", bufs=2, space="PSUM"))

        ones_r = cpool.tile([1, 128], F32, name="tl", tag="ones_r")
        nc.gpsimd.memset(ones_r[:], 1.0)
        ones64 = cpool.tile([128, 64], F32, name="tl", tag="ones64")
        nc.gpsimd.memset(ones64[:], 1.0)
        ones64b = cpool.tile([128, 64], BF16, name="tl", tag="ones64b")
        nc.gpsimd.memset(ones64b[:], 1.0)
        eps_c = cpool.tile([1, 1], F32, name="tl", tag="eps_c")
        nc.gpsimd.memset(eps_c[:], 1e-5)

        # initial descriptor state: one DMA
        nc.sync.dma_start(dstb[:, :, :, :], xcb[:, :, :, :])

        qpads, qp_ctr = [], [0]
        if not SC_ROWSPLIT:
            for bi in range(2):
                qp = [cpool.tile([128, 256], BF16, name="tl", tag=f"qpad{bi}_{c}")
                      for c in range(2)]
                for c in range(2):
                    nc.gpsimd.memset(qp[c][:], 0.0)
                qpads.append(qp)

        def load_bias(src_ap, n, tag):
            t_ = bp.tile([128, n // 128], F32, name="tl", tag=tag)
            nc.sync.dma_start(t_[:], src_ap.rearrange("(a p) -> p a", p=128))
            return t_

        for i in [li for _r in range(reps) for li in range(n_layers)]:
            props = LAYER_PROPS[i]
            lns = LAYER_LNS[i]
            g0, g1 = grp_split(props)
            gidx = {}
            for g, idxs in enumerate([g0, g1]):
                for k, j in enumerate(idxs):
                    gidx[j] = (g, k)
            brs = sorted(set(p[0] for p in props))

            # ---- prefetch packed weights
            bpk, apk = {}, {}
            for br in brs:
                t = wbp.tile([128, 4096], BF16, name="tl", tag="bpk")
                nc.sync.dma_start(t[:], wpackB[i, br])
                bpk[br] = t
                t = wap.tile([128, 1024], BF16, name="tl", tag="apk")
                nc.sync.dma_start(t[:], wpackA[i, br])
                apk[br] = t

            if not zb:
                bk_t = {br: load_bias(pbk[i, br], 256, "bk") for br in brs}
                bq_t = {br: load_bias(pbq[i, br], 256, "bq") for br in brs}
                mb_t = {br: load_bias(mbb[i, br], 256, "mb") for br in brs}
                b1_t = {br: load_bias(b1b[i, br], 512, "b1") for br in brs}
                b2_t = {br: load_bias(b2b[i, br], 256, "b2") for br in brs}
                bv_b = {}
                for br in brs:
                    bvr = sp_.tile([1, 256], F32, name="tl", tag="bvr")
                    nc.sync.dma_start(bvr[:], pbv[i, br][None, :])
                    bv_ps = psp.tile([128, 256], F32, name="tl", tag="ps")
                    nc.tensor.matmul(bv_ps[:], ones_r[:], bvr[:], start=True, stop=True)
                    t = sp_.tile([128, 256], F32, name="tl", tag="bvb")
                    nc.vector.tensor_copy(t[:], bv_ps[:])
                    bv_b[br] = t

            # ---- stage A: local k/v chunks per pair, AllGather per group
            for g, idxs in enumerate([g0, g1]):
                if not idxs or "A" not in stages:
                    continue
                for k, j in enumerate(idxs):
                    br, xi, si = props[j]
                    a = apk[br]
                    kv_sb = kvwp.tile([128, 512], BF16, name="tl", tag="kvw")
                    for c in range(2):
                        kc_ps = psp.tile([128, 512], F32, name="tl", tag="ps")
                        for cc in range(2):
                            nc.tensor.matmul(
                                kc_ps[:, 0:CH],
                                a[:, cc * 256 + c * 128:cc * 256 + c * 128 + 128],
                                dstb[:, si, cc, :], start=(cc == 0), stop=(cc == 1))
                        if zb:
                            nc.vector.tensor_copy(kv_sb[:, c * 128:(c + 1) * 128],
                                                  kc_ps[:, 0:CH])
                        else:
                            nc.scalar.activation(kv_sb[:, c * 128:(c + 1) * 128],
                                                 kc_ps[:, 0:CH], AF.Identity,
                                                 bias=bk_t[br][:, c:c + 1])
                    vt_ps = psp.tile([128, 512], F32, name="tl", tag="ps")
                    for cc in range(2):
                        nc.tensor.matmul(vt_ps[:, 0:256], dstb[:, si, cc, :],
                                         a[:, 512 + cc * 256:512 + (cc + 1) * 256],
                                         start=(cc == 0), stop=(cc == 1))
                    if zb:
                        nc.vector.tensor_copy(kv_sb[:, 256:512], vt_ps[:, 0:256])
                    else:
                        nc.vector.tensor_add(kv_sb[:, 256:512], vt_ps[:, 0:256],
                                             bv_b[br][:])
                    nc.sync.dma_start(agin_t[i][g][k], kv_sb[:])
                nc.gpsimd.collective_compute(
                    "AllGather", mybir.AluOpType.bypass, replica_groups=rg,
                    ins=[agin_t[i][g].ap().opt()], outs=[agout_t[i][g].ap().opt()])

            # ---- stage B: per-prop attention + merge + MLP on local queries
            ndelta = {t: sum(1 for p in props if p[1] == t) for t in range(4)}
            seen = {t: 0 for t in range(4)}
            first_delta = {t: True for t in range(4)}
            ln_done = set()

            def emit_ln(t):
                # residual + LayerNorm over channel dim (partitions x 2 ctiles)
                if "C" not in stages:
                    return
                xq = lnp.tile([128, 1024], F32, name="tl", tag="xq")
                s_ps = psp.tile([128, 512], F32, name="tl", tag="ps")
                for c in range(2):
                    nc.vector.tensor_add(xq[:, c * 256:c * 256 + CH],
                                         dstb[:, t, c, :],
                                         dlt[:, t, c * CH:(c + 1) * CH])
                    nc.vector.tensor_mul(xq[:, c * 256 + CH:c * 256 + 2 * CH],
                                         xq[:, c * 256:c * 256 + CH],
                                         xq[:, c * 256:c * 256 + CH])
                    nc.tensor.matmul(s_ps[0:64, 0:256], ones64[:],
                                     xq[:, c * 256:(c + 1) * 256],
                                     start=(c == 0), stop=(c == 1))
                st = sp_.tile([1, 512], F32, name="tl", tag="st")
                nc.vector.tensor_scalar_mul(st[:, 0:CH], s_ps[0:1, 0:CH], 1.0 / 256)
                nc.vector.tensor_scalar_mul(st[:, CH:256], s_ps[0:1, CH:256], 1.0 / 256)
                nc.vector.tensor_mul(st[:, 256:384], st[:, 0:CH], st[:, 0:CH])
                nc.vector.tensor_sub(st[:, 256:384], st[:, CH:256], st[:, 256:384])
                # rs = (var+eps)^-0.5 via ln/exp (stays in the exp table set)
                nc.scalar.activation(st[:, 384:512], st[:, 256:384], AF.Ln,
                                     bias=eps_c[:])
                nc.scalar.activation(st[:, 256:384], st[:, 384:512], AF.Exp,
                                     scale=-0.5)
                bc_ps = psp.tile([128, 512], F32, name="tl", tag="ps")
                nc.tensor.matmul(bc_ps[:, 0:CH], ones_r[:], st[:, 0:CH],
                                 start=True, stop=True)
                nc.tensor.matmul(bc_ps[:, CH:256], ones_r[:], st[:, 256:384],
                                 start=True, stop=True)
                if not ln_triv:
                    g_row = sp_.tile([1, 256], F32, name="tl", tag="grow")
                    nc.sync.dma_start(g_row[:], lng[i, t][None, :])
                    b_col = bp.tile([128, 2], F32, name="tl", tag="lnb")
                    nc.sync.dma_start(b_col[:], lnb[i, t].rearrange("(a p) -> p a", p=128))
                bc_sb = abp.tile([128, 256], F32, name="tl", tag="bcln")
                nc.vector.tensor_copy(bc_sb[:], bc_ps[:, 0:256])
                for c in range(2):
                    tmp = tmpp.tile([128, CH], F32, name="tl", tag="tmp")
                    nc.vector.tensor_sub(tmp[:], xq[:, c * 256:c * 256 + CH],
                                         bc_sb[:, 0:CH])
                    if ln_triv:
                        nc.vector.tensor_mul(dstb[:, t, c, :], tmp[:],
                                             bc_sb[:, CH:256])
                    else:
                        g_ps = psp.tile([128, 512], F32, name="tl", tag="ps")
                        nc.tensor.matmul(g_ps[:, 0:CH],
                                         g_row[:, c * 128:(c + 1) * 128],
                                         st[:, 256:384], start=True, stop=True)
                        gs = tmpp.tile([128, CH], F32, name="tl", tag="gs")
                        nc.vector.tensor_copy(gs[:], g_ps[:, 0:CH])
                        nc.vector.tensor_mul(tmp[:], tmp[:], gs[:])
                        nc.vector.tensor_scalar_add(dstb[:, t, c, :], tmp[:],
                                                    b_col[:, c:c + 1])

            for j in (g0 + g1 if "B" in stages else []):
                br, xi, si = props[j]
                g, k = gidx[j]
                b = bpk[br]
                kvg = kvgp.tile([128, NC * 512], BF16, name="tl", tag="kvg")
                nc.sync.dma_start(
                    kvg[:].rearrange("p (r f) -> p r f", r=NC),
                    agout_t[i][g].ap().rearrange("r q p f -> q p r f")[k])

                # q projection -> per-ctile head-stacked tiles (bf16)
                qh = [qhp.tile([128, CH], BF16, name="tl", tag="qh") for _ in range(2)]
                if not SC_ROWSPLIT:
                    qh = qpads[qp_ctr[0] % 2]
                    qp_ctr[0] += 1
                for c in range(2):
                    q_ps = psp.tile([128, 512], F32, name="tl", tag="ps")
                    for cc in range(2):
                        nc.tensor.matmul(
                            q_ps[:, 0:CH],
                            b[:, cc * 256 + c * 128:cc * 256 + c * 128 + 128],
                            dstb[:, xi, cc, :], start=(cc == 0), stop=(cc == 1))
                    if not SC_ROWSPLIT:
                        nc.vector.tensor_copy(qh[c][0:64, 0:CH], q_ps[0:64, 0:CH])
                        nc.vector.tensor_copy(qh[c][64:128, CH:2 * CH],
                                              q_ps[64:128, 0:CH])
                    elif zb:
                        nc.vector.tensor_copy(qh[c][:], q_ps[:, 0:CH])
                    else:
                        nc.scalar.activation(qh[c][:], q_ps[:, 0:CH], AF.Identity,
                                             bias=bq_t[br][:, c:c + 1])

                if blvl < 2:
                    continue
                # scores^T (keys x [h,q]) via two concurrent K=64 row-tiles; exp
                e_t = []
                for mp in range(NC // 2):
                    sc_ps = scp.tile([128, 1024], F32, name="tl", tag="sc")
                    for ml in range(2):
                        m = 2 * mp + ml
                        for c in range(2):
                            if not SC_ROWSPLIT:
                                nc.tensor.matmul(
                                    sc_ps[:, ml * 512 + (2 * c) * CH:
                                          ml * 512 + (2 * c + 2) * CH],
                                    kvg[:, m * 512 + c * 128:m * 512 + (c + 1) * 128],
                                    qh[c][:], start=True, stop=True)
                                continue
                            for hh in range(2):
                                nc.tensor.matmul(
                                    sc_ps[:, ml * 512 + (2 * c + hh) * CH:
                                          ml * 512 + (2 * c + hh + 1) * CH],
                                    kvg[64 * hh:64 * hh + 64,
                                        m * 512 + c * 128:m * 512 + (c + 1) * 128],
                                    qh[c][64 * hh:64 * hh + 64, :],
                                    start=True, stop=True,
                                    tile_position=(64 * hh, 0))
                    e_sb = ep.tile([128, 1024], BF16, name="tl", tag="exps")
                    if EXP_SPLIT:
                        nc.scalar.activation(e_sb[:, 0:512], sc_ps[:, 0:512], AF.Exp)
                        nc.scalar.activation(e_sb[:, 512:1024], sc_ps[:, 512:1024],
                                             AF.Exp)
                    else:
                        nc.scalar.activation(e_sb[:], sc_ps[:], AF.Exp)
                    e_t.append(e_sb)

                # Z accumulation (DVE, bf16)
                if blvl >= 3:
                    z_acc = zap.tile([128, 512], BF16, name="tl", tag="zacc")
                    nc.vector.tensor_copy(z_acc[:], e_t[0][:, 0:512])
                    for m in range(1, NC):
                        nc.vector.tensor_add(
                            z_acc[:], z_acc[:],
                            e_t[m // 2][:, (m % 2) * 512:(m % 2) * 512 + 512])

                # A @ V accumulated over m-tiles (one PSUM bank per ctile)
                av_ps = [avp.tile([128, CH], F32, name="tl", tag=f"av{c}")
                         for c in range(2)]
                for m in (range(NC) if blvl >= 4 else []):
                    e_sl = e_t[m // 2]
                    off = (m % 2) * 512
                    for h in range(H):
                        c, o = h // 2, 64 * (h % 2)
                        nc.tensor.matmul(
                            av_ps[c][o:o + 64, :],
                            kvg[:, m * 512 + 256 + 64 * h:m * 512 + 256 + 64 * h + 64],
                            e_sl[:, off + h * CH:off + (h + 1) * CH],
                            start=(m == 0), stop=(m == NC - 1),
                            tile_position=(0, o), skip_group_check=True)

                if blvl < 5:
                    continue
                # normalize: attn = av * (1/Z) broadcast
                z_ps = psp.tile([128, 512], F32, name="tl", tag="ps")
                nc.tensor.matmul(z_ps[0:64, :], ones64b[:], z_acc[:],
                                 start=True, stop=True)
                r_row = sp_.tile([1, 512], F32, name="tl", tag="rz")
                nc.vector.reciprocal(r_row[:], z_ps[0:1, :])
                bc_ps = psp.tile([128, 512], F32, name="tl", tag="ps")
                for h in range(H):
                    c, o = h // 2, 64 * (h % 2)
                    nc.tensor.matmul(bc_ps[o:o + 64, c * CH:(c + 1) * CH],
                                     ones_r[:, 0:64], r_row[:, h * CH:(h + 1) * CH],
                                     start=True, stop=True, tile_position=(0, o))
                bc_sb = abp.tile([128, 256], F32, name="tl", tag="bcav")
                nc.vector.tensor_copy(bc_sb[:], bc_ps[:, 0:256])
                attn_sb = atp.tile([128, 256], BF16, name="tl", tag="attn")
                for c in range(2):
                    nc.vector.tensor_mul(attn_sb[:, c * CH:(c + 1) * CH],
                                         av_ps[c][:], bc_sb[:, c * CH:(c + 1) * CH])

                if blvl < 6:
                    continue
                # merge
                m_ps = psp.tile([128, 512], F32, name="tl", tag="ps")
                for c in range(2):
                    for cc in range(2):
                        nc.tensor.matmul(
                            m_ps[:, c * CH:(c + 1) * CH],
                            b[:, 512 + cc * 256 + c * 128:512 + cc * 256 + c * 128 + 128],
                            attn_sb[:, cc * CH:(cc + 1) * CH],
                            start=(cc == 0), stop=(cc == 1))
                msg_sb = msp.tile([128, 256], BF16, name="tl", tag="msg")
                if zb:
                    nc.vector.tensor_copy(msg_sb[:], m_ps[:, 0:256])
                else:
                    for c in range(2):
                        nc.scalar.activation(msg_sb[:, c * CH:(c + 1) * CH],
                                             m_ps[:, c * CH:(c + 1) * CH],
                                             AF.Identity, bias=mb_t[br][:, c:c + 1])

                if blvl < 7:
                    continue
                # mlp1 (relu) on concat([x, msg]); two c's share one PSUM bank
                def h_in(cc):
                    if cc < 2:
                        return dstb[:, xi, cc, :]
                    return msg_sb[:, (cc - 2) * CH:(cc - 1) * CH]
                h1_sb = []
                for pair in range(2):
                    h_ps = psp.tile([128, 512], F32, name="tl", tag="ps")
                    for cl in range(2):
                        c = pair * 2 + cl
                        for cc in range(4):
                            nc.tensor.matmul(
                                h_ps[:, cl * CH:(cl + 1) * CH],
                                b[:, 1024 + cc * 512 + c * 128:
                                  1024 + cc * 512 + c * 128 + 128],
                                h_in(cc), start=(cc == 0), stop=(cc == 3))
                    t_ = h1p.tile([128, 256], BF16, name="tl", tag="h1")
                    if zb:
                        nc.vector.tensor_relu(t_[:], h_ps[:, 0:256])
                    else:
                        for cl in range(2):
                            c = pair * 2 + cl
                            nc.scalar.activation(t_[:, cl * CH:(cl + 1) * CH],
                                                 h_ps[:, cl * CH:(cl + 1) * CH],
                                                 AF.Relu, bias=b1_t[br][:, c:c + 1])
                    h1_sb.append(t_)

                if blvl < 8:
                    continue
                # mlp2 -> delta accumulation
                d_ps = psp.tile([128, 512], F32, name="tl", tag="ps")
                for c in range(2):
                    for cc in range(4):
                        nc.tensor.matmul(
                            d_ps[:, c * CH:(c + 1) * CH],
                            b[:, 3072 + cc * 256 + c * 128:
                              3072 + cc * 256 + c * 128 + 128],
                            h1_sb[cc // 2][:, (cc % 2) * CH:(cc % 2 + 1) * CH],
                            start=(cc == 0), stop=(cc == 3))
                if zb:
                    if first_delta[xi]:
                        nc.vector.tensor_copy(dlt[:, xi, :], d_ps[:, 0:256])
                    else:
                        nc.vector.tensor_add(dlt[:, xi, :], dlt[:, xi, :],
                                             d_ps[:, 0:256])
                else:
                    tmp = tmpp.tile([128, 256], F32, name="tl", tag="dtmp")
                    for c in range(2):
                        nc.scalar.activation(tmp[:, c * CH:(c + 1) * CH],
                                             d_ps[:, c * CH:(c + 1) * CH],
                                             AF.Identity, bias=b2_t[br][:, c:c + 1])
                    if first_delta[xi]:
                        nc.vector.tensor_copy(dlt[:, xi, :], tmp[:])
                    else:
                        nc.vector.tensor_add(dlt[:, xi, :], dlt[:, xi, :], tmp[:])
                first_delta[xi] = False
                seen[xi] += 1
                if seen[xi] == ndelta[xi] and xi in lns and xi not in ln_done:
                    ln_done.add(xi)
                    emit_ln(xi)

            if "C" in stages and "B" not in stages:
                for t in lns:
                    emit_ln(t)

        # ---- epilogue: out[m] = (1/32) qvec^T kmat[:, m]
        s1 = sp_.tile([128, 2], F32, name="tl", tag="s1")
        for c in range(2):
            nc.vector.reduce_sum(s1[:, c:c + 1], dstb[:, 1, c, :],
                                 axis=mybir.AxisListType.X)
            nc.sync.dma_start(ag2in[c], s1[:, c:c + 1])
        nc.gpsimd.collective_compute(
            "AllGather", mybir.AluOpType.bypass, replica_groups=rg,
            ins=[ag2in.ap().opt()], outs=[ag2out.ap().opt()])
        gath = sp_.tile([128, NC], F32, name="tl", tag="gath")
        d1b = sp_.tile([128, 2], F32, name="tl", tag="d1b")
        d1bb = sp_.tile([128, 2], BF16, name="tl", tag="d1bb")
        for c in range(2):
            nc.sync.dma_start(gath[:], ag2out.ap().rearrange("r c p o -> c p (r o)")[c])
            nc.vector.reduce_sum(d1b[:, c:c + 1], gath[:], axis=mybir.AxisListType.X)
        nc.vector.tensor_copy(d1bb[:], d1b[:])

        wq5 = [cpool.tile([128, 256], BF16, name="tl", tag=f"wq5{k}") for k in range(2)]
        wk5 = [cpool.tile([128, 256], BF16, name="tl", tag=f"wk5{k}") for k in range(2)]
        for k in range(2):
            nc.sync.dma_start(wq5[k][:], w5T[0, k * 128:(k + 1) * 128, :])
            nc.sync.dma_start(wk5[k][:], w5T[1, k * 128:(k + 1) * 128, :])
        b5 = bp.tile([128, 4], F32, name="tl", tag="b5")
        nc.sync.dma_start(b5[:], pb5.rearrange("t (a p) -> p (t a)", p=128))
        qv = sp_.tile([128, 2], F32, name="tl", tag="qv")
        for c in range(2):
            q_ps = psp.tile([128, 512], F32, name="tl", tag="ps")
            for cc in range(2):
                nc.tensor.matmul(q_ps[:, 0:1], wq5[cc][:, c * 128:(c + 1) * 128],
                                 d1bb[:, cc:cc + 1], start=(cc == 0), stop=(cc == 1))
            nc.scalar.activation(qv[:, c:c + 1], q_ps[:, 0:1], AF.Identity,
                                 bias=b5[:, c:c + 1], scale=1.0 / N)
        km = [sp_.tile([128, CH], F32, name="tl", tag=f"km{c}") for c in range(2)]
        for c in range(2):
            k_ps = psp.tile([128, 512], F32, name="tl", tag="ps")
            for cc in range(2):
                nc.tensor.matmul(k_ps[:, 0:CH], wk5[cc][:, c * 128:(c + 1) * 128],
                                 dstb[:, 0, cc, :], start=(cc == 0), stop=(cc == 1))
            nc.scalar.activation(km[c][:], k_ps[:, 0:CH], AF.Identity,
                                 bias=b5[:, 2 + c:3 + c])
        o_ps = psp.tile([128, 512], F32, name="tl", tag="ps")
        for c in range(2):
            nc.vector.tensor_scalar_mul(km[c][:], km[c][:], qv[:, c:c + 1])
            nc.tensor.matmul(o_ps[0:64, 0:CH], ones64[:], km[c][:],
                             start=(c == 0), stop=(c == 1))
        o_sb = sp_.tile([1, CH], F32, name="tl", tag="osb")
        nc.scalar.activation(o_sb[:], o_ps[0:1, 0:CH], AF.Copy, scale=1.0 / 32)
        nc.sync.dma_start(out_d[:], o_sb[:])

    nc.compile()
    return nc


def prep_inputs(inputs, zb=True, ln_triv=True):
    inp = {k: np.ascontiguousarray(np.asarray(v)) for k, v in inputs.items()}
    pw, pb = inp['proj_w'].astype(np.float32), inp['proj_b'].astype(np.float32)
    mw = inp['merge_w'].astype(np.float32)
    w1 = inp['mlp_w1'].astype(np.float32)
    w2 = inp['mlp_w2'].astype(np.float32)

    bf = mybir.dt.np(mybir.dt.bfloat16)
    wpackA = np.empty((5, 3, 128, 1024), np.float32)
    wpackB = np.empty((5, 3, 128, 4096), np.float32)
    pbq = np.empty((5, 3, 256), np.float32)
    pbk = np.empty((5, 3, 256), np.float32)
    pbv = np.empty((5, 3, 256), np.float32)
    for i in range(5):
        for br in range(3):
            wqT = pw[br, i, 0][PERM].T * 0.125   # [256 in, 256 out]
            wkT = pw[br, i, 1][PERM].T
            wvT = pw[br, i, 2][PERM].T
            mgT = mw[br, i][:, PERM].T
            w1T = w1[br, i].T                     # [512, 512]
            w2T = w2[br, i].T                     # [512, 256]
            wpackA[i, br, :, 0:256] = wkT[0:128]
            wpackA[i, br, :, 256:512] = wkT[128:256]
            wpackA[i, br, :, 512:768] = wvT[0:128]
            wpackA[i, br, :, 768:1024] = wvT[128:256]
            wpackB[i, br, :, 0:256] = wqT[0:128]
            wpackB[i, br, :, 256:512] = wqT[128:256]
            wpackB[i, br, :, 512:768] = mgT[0:128]
            wpackB[i, br, :, 768:1024] = mgT[128:256]
            for cc in range(4):
                wpackB[i, br, :, 1024 + cc * 512:1024 + (cc + 1) * 512] = \
                    w1T[cc * 128:(cc + 1) * 128]
                wpackB[i, br, :, 3072 + cc * 256:3072 + (cc + 1) * 256] = \
                    w2T[cc * 128:(cc + 1) * 128]
            pbq[i, br] = pb[br, i, 0][PERM] * 0.125
            pbk[i, br] = pb[br, i, 1][PERM]
            pbv[i, br] = pb[br, i, 2][PERM]

    w5T = np.stack([pw[0, 5, 0].T, pw[0, 5, 1].T]).astype(bf)
    pb5 = np.stack([pb[0, 5, 0], pb[0, 5, 1]]).astype(np.float32)

    desc = np.stack([inp[f'desc{t}'][0] for t in range(4)]).astype(np.float32)
    shared = dict(wpackA=wpackA.astype(bf), wpackB=wpackB.astype(bf),
                  w5T=w5T, pb5=pb5)
    if not zb:
        mb = inp['merge_b'].astype(np.float32)
        b1 = inp['mlp_b1'].astype(np.float32)
        b2 = inp['mlp_b2'].astype(np.float32)
        shared.update(
            pbq=pbq, pbk=pbk, pbv=pbv,
            mbb=np.ascontiguousarray(np.transpose(mb[:, :5], (1, 0, 2))),
            b1b=np.ascontiguousarray(np.transpose(b1[:, :5], (1, 0, 2))),
            b2b=np.ascontiguousarray(np.transpose(b2[:, :5], (1, 0, 2))))
    if not ln_triv:
        ng = inp['norm_g'].astype(np.float32)
        nb = inp['norm_b'].astype(np.float32)
        shared.update(
            lng=np.ascontiguousarray(np.transpose(ng[:, :5], (1, 0, 2))),
            lnb=np.ascontiguousarray(np.transpose(nb[:, :5], (1, 0, 2))))

    in_maps = []
    for j in range(NC):
        xcj = desc[:, :, j * CH:(j + 1) * CH].reshape(4, 2, 128, CH)
        xcb = np.ascontiguousarray(np.transpose(xcj, (2, 0, 1, 3))).astype(bf)
        in_maps.append({"xcb": xcb, **shared})
    return in_maps


def kernel(**inputs):
    zb = all(not np.asarray(inputs[k]).any() for k in
             ("proj_b", "merge_b", "mlp_b1", "mlp_b2"))
    ln_triv = (np.asarray(inputs["norm_g"])[:, :5] == 1).all() and \
        not np.asarray(inputs["norm_b"])[:, :5].any()
    key = f"nc{zb}_{ln_triv}"
    if key not in _cache:
        _cache[key] = build_kernel(zb=zb, ln_triv=ln_triv)
    nc = _cache[key]
    in_maps = prep_inputs(inputs, zb=zb, ln_triv=ln_triv)
    res = run_bass_kernel_spmd(nc, in_maps, core_ids=list(range(NC)))
    out = np.concatenate([res.results[j]["out"][0] for j in range(NC)])
    mask = np.asarray(inputs["unreachable"]).any(axis=0)
    out = np.where(mask, np.float32(-1e9), out.astype(np.float32))
    return out


# revision 30
# speedup vs baseline: 1.7213x; 1.7213x over previous
"""Trainium2 Bass kernel for nn_AttentionalGNN (8-core SPMD, sequence-sharded) v2.

Strategy (vs v1 baseline):
  - N=1024 node axis sharded over 8 cores (CH=128 queries each); k/v computed
    locally per (branch, source) pair and shared via AllGather (split into two
    collectives per layer: br0 first so stage B can start while the rest fly).
  - All weights of one (layer, branch) packed host-side into single DRAM blobs
    -> one DMA each (was ~14).  k/v chunks packed [128, 512] per pair -> one
    write DMA + one 1MB gather-read DMA per prop (was ~10 small ones).
  - Dead code pruned: layer-3 d2/d3 updates, layer-4 d2/d3 props + LNs are
    never observable in the output (layer 5 only needs d0/d1).
  - Single ACT table set (natural_log_exp_and_others): LN rsqrt is computed as
    exp(-0.5*ln(var+eps)) instead of Sqrt, so the exp table never reloads.
  - Scores via two concurrent K=64 row-tiles (heads packed in partitions),
    exp over [128,1024] 2-bank PSUM tiles (amortizes ACT per-op overhead).
  - d state kept in bf16; residual+LN fused ops; per-prop DVE op count cut.
  - Final layer-5 score collapses to two projections + tiny AllGather
    (out[m] = (1/32)(Wq5 @ mean_n d1)^T (Wk5 @ d0)[:, m]).
"""
import numpy as np

import concourse.bass as bass
import concourse.bacc as bacc
import concourse.mybir as mybir
import concourse.tile as tile
from concourse.bass_utils import run_bass_kernel_spmd

D, N, H, DH = 256, 1024, 4, 64
NC = 8
CH = N // NC  # 128 positions per core
F32 = mybir.dt.float32
BF16 = mybir.dt.bfloat16
AF = mybir.ActivationFunctionType

# props per layer (br, xi, si), dead ones pruned (output only needs d0/d1
# after layer 4; layer-5 'cross' collapses into the epilogue).
LAYER_PROPS = [
    [(0, 0, 0), (0, 1, 1), (1, 2, 2), (2, 3, 3)],
    [(0, 0, 1), (0, 1, 0), (1, 2, 1), (1, 1, 2), (2, 0, 3), (2, 3, 0)],
    [(0, 0, 0), (0, 1, 1), (1, 2, 2), (2, 3, 3)],
    [(0, 0, 1), (0, 1, 0), (1, 1, 2), (2, 0, 3)],
    [(0, 0, 0), (0, 1, 1)],
]
LAYER_LNS = [[0, 1, 2, 3], [0, 1, 2, 3], [0, 1, 2, 3], [0, 1], [0, 1]]
PERM = np.array([4 * (r % 64) + r // 64 for r in range(256)])

_cache = {}
import os as _os
EXP_SPLIT = _os.environ.get("K_EXP_SPLIT", "0") == "1"
SC_ROWSPLIT = _os.environ.get("K_SC_ROWSPLIT", "0") == "1"

# The act-table picker binds each function to the first table set containing
# it, so Exp->exp_and_others while Ln->natural_log_exp_and_others, and every
# LayerNorm causes two ~2.7us table reloads between softmax exps.  Removing
# Exp/Copy/Identity from the sets that lack Ln (dict order and hence set ids
# are preserved; natural_log_exp_and_others genuinely contains all of them)
# makes every activation resolve to that one set -> a single table load.
import functools as _functools
import concourse.hw_specs as _hw_specs
import concourse.bacc as _bacc_mod
_orig_gat = _hw_specs.get_activation_tables


@_functools.cache
def _gat_one_set(arch):
    tabs = _orig_gat(arch)
    keep = "natural_log_exp_and_others"
    if keep not in tabs:
        return tabs
    shared = tabs[keep]
    out = {}
    for name, fns in tabs.items():
        out[name] = fns if name == keep else (set(fns) - shared)
    return out


_hw_specs.get_activation_tables = _gat_one_set
_bacc_mod.get_activation_tables = _gat_one_set


def build_kernel(trace_scopes=False, n_layers=5, stages="ABC", blvl=9, reps=1,
                 zb=True, ln_triv=True):
    nc = bacc.Bacc("TRN2", target_bir_lowering=False, debug=False, num_devices=NC)

    # ---- I/O ----
    xcb = nc.dram_tensor("xcb", [128, 4, 2, CH], BF16, kind="ExternalInput")
    wpackA = nc.dram_tensor("wpackA", [5, 3, 128, 1024], BF16, kind="ExternalInput")
    wpackB = nc.dram_tensor("wpackB", [5, 3, 128, 4096], BF16, kind="ExternalInput")
    w5T = nc.dram_tensor("w5T", [2, 256, 256], BF16, kind="ExternalInput")  # [qT,kT]
    pb5 = nc.dram_tensor("pb5", [2, 256], F32, kind="ExternalInput")
    if not zb:
        pbq = nc.dram_tensor("pbq", [5, 3, 256], F32, kind="ExternalInput")
        pbk = nc.dram_tensor("pbk", [5, 3, 256], F32, kind="ExternalInput")
        pbv = nc.dram_tensor("pbv", [5, 3, 256], F32, kind="ExternalInput")
        mbb = nc.dram_tensor("mbb", [5, 3, 256], F32, kind="ExternalInput")
        b1b = nc.dram_tensor("b1b", [5, 3, 512], F32, kind="ExternalInput")
        b2b = nc.dram_tensor("b2b", [5, 3, 256], F32, kind="ExternalInput")
    if not ln_triv:
        lng = nc.dram_tensor("lng", [5, 4, 256], F32, kind="ExternalInput")
        lnb = nc.dram_tensor("lnb", [5, 4, 256], F32, kind="ExternalInput")
    out_d = nc.dram_tensor("out", [1, CH], F32, kind="ExternalOutput")

    # per-(layer, group) collective buffers
    def grp_split(props):
        g0 = [j for j, p in enumerate(props) if p[0] == 0]
        g1 = [j for j, p in enumerate(props) if p[0] != 0]
        return g0, g1

    agin_t, agout_t = [], []
    for i in range(5):
        g0, g1 = grp_split(LAYER_PROPS[i])
        ins, outs = [], []
        for g, idxs in enumerate([g0, g1]):
            if idxs:
                ins.append(nc.dram_tensor(f"agin{i}_{g}", [len(idxs), 128, 512], BF16))
                outs.append(nc.dram_tensor(f"agout{i}_{g}", [NC, len(idxs), 128, 512],
                                           BF16, addr_space="Shared"))
            else:
                ins.append(None)
                outs.append(None)
        agin_t.append(ins)
        agout_t.append(outs)
    ag2in = nc.dram_tensor("ag2in", [2, 128, 1], F32)
    ag2out = nc.dram_tensor("ag2out", [NC, 2, 128, 1], F32, addr_space="Shared")

    # persistent SBUF state (bf16 d-state; f32 delta accumulator)
    dstb = nc.alloc_sbuf_tensor("dstateb", [128, 4, 2, CH], BF16)
    dlt = nc.alloc_sbuf_tensor("delta", [128, 4, 2 * CH], F32)

    rg = [list(range(NC))]

    from contextlib import ExitStack
    with ExitStack() as es:
        tc = es.enter_context(tile.TileContext(nc))
        cpool = es.enter_context(tc.tile_pool(name="const", bufs=1))
        wbp = es.enter_context(tc.tile_pool(name="wb", bufs=6))
        wap = es.enter_context(tc.tile_pool(name="wa", bufs=6))
        kvwp = es.enter_context(tc.tile_pool(name="kvw", bufs=4))
        kvgp = es.enter_context(tc.tile_pool(name="kvg", bufs=4))
        qhp = es.enter_context(tc.tile_pool(name="qh", bufs=6))
        ep = es.enter_context(tc.tile_pool(name="exps", bufs=9))
        ztp = es.enter_context(tc.tile_pool(name="ztree", bufs=3))
        zap = es.enter_context(tc.tile_pool(name="zacc", bufs=2))
        sp_ = es.enter_context(tc.tile_pool(name="small", bufs=4))
        abp = es.enter_context(tc.tile_pool(name="bcsb", bufs=2))
        atp = es.enter_context(tc.tile_pool(name="attn", bufs=2))
        msp = es.enter_context(tc.tile_pool(name="msg", bufs=2))
        h1p = es.enter_context(tc.tile_pool(name="h1", bufs=4))
        lnp = es.enter_context(tc.tile_pool(name="ln", bufs=2))
        tmpp = es.enter_context(tc.tile_pool(name="tmp", bufs=2))
        bp = es.enter_context(tc.tile_pool(name="bias", bufs=3))
        scp = es.enter_context(tc.tile_pool(name="sc", bufs=2, space="PSUM"))
        avp = es.enter_context(tc.tile_pool(name="av", bufs=1, space="PSUM"))
        psp = es.enter_context(tc.tile_pool(name="ps", bufs=2, space="PSUM"))

        ones_r = cpool.tile([1, 128], F32, name="tl", tag="ones_r")
        nc.gpsimd.memset(ones_r[:], 1.0)
        ones64 = cpool.tile([128, 64], F32, name="tl", tag="ones64")
        nc.gpsimd.memset(ones64[:], 1.0)
        ones64b = cpool.tile([128, 64], BF16, name="tl", tag="ones64b")
        nc.gpsimd.memset(ones64b[:], 1.0)
        eps_c = cpool.tile([1, 1], F32, name="tl", tag="eps_c")
        nc.gpsimd.memset(eps_c[:], 1e-5)

        # initial descriptor state: one DMA
        nc.sync.dma_start(dstb[:, :, :, :], xcb[:, :, :, :])

        qpads, qp_ctr = [], [0]
        if not SC_ROWSPLIT:
            for bi in range(2):
                qp = [cpool.tile([128, 256], BF16, name="tl", tag=f"qpad{bi}_{c}")
                      for c in range(2)]
                for c in range(2):
                    nc.gpsimd.memset(qp[c][:], 0.0)
                qpads.append(qp)

        def load_bias(src_ap, n, tag):
            t_ = bp.tile([128, n // 128], F32, name="tl", tag=tag)
            nc.sync.dma_start(t_[:], src_ap.rearrange("(a p) -> p a", p=128))
            return t_

        seq = [li for _r in range(reps) for li in range(n_layers)]

        def new_actx(li):
            """Stage-A context for layer li: apk prefetch + pending pair table."""
            props_ = LAYER_PROPS[li]
            g0_, g1_ = grp_split(props_)
            pairs = []
            for g, idxs in enumerate([g0_, g1_]):
                for k, j in enumerate(idxs):
                    br, _, si = props_[j]
                    pairs.append((g, k, br, si))
            brs_ = sorted(set(p[0] for p in props_))
            apk = {}
            for br in brs_:
                t = wap.tile([128, 1024], BF16, name="tl", tag="apk")
                nc.sync.dma_start(t[:], wpackA[li, br])
                apk[br] = t
            ctx = dict(i=li, pairs=pairs, apk=apk,
                       left=[len(g0_), len(g1_)], done=set())
            if not zb:
                ctx["bk"] = {br: load_bias(pbk[li, br], 256, "bk") for br in brs_}
                bv_b = {}
                for br in brs_:
                    bvr = sp_.tile([1, 256], F32, name="tl", tag="bvr")
                    nc.sync.dma_start(bvr[:], pbv[li, br][None, :])
                    bv_ps = psp.tile([128, 512], F32, name="tl", tag="ps")
                    nc.tensor.matmul(bv_ps[:, 0:256], ones_r[:], bvr[:],
                                     start=True, stop=True)
                    t = sp_.tile([128, 256], F32, name="tl", tag="bvb")
                    nc.vector.tensor_copy(t[:], bv_ps[:, 0:256])
                    bv_b[br] = t
                ctx["bv"] = bv_b
            return ctx

        def emit_A_for(actx, t=None):
            """kv-proj + agin write for actx pairs sourcing t (None = all);
            fires the group AllGather once its last pair is written."""
            if actx is None or "A" not in stages:
                return
            li = actx["i"]
            for (g, k, br, si) in actx["pairs"]:
                if (g, k) in actx["done"] or (t is not None and si != t):
                    continue
                actx["done"].add((g, k))
                a = actx["apk"][br]
                kv_sb = kvwp.tile([128, 512], BF16, name="tl", tag="kvw")
                for c in range(2):
                    kc_ps = psp.tile([128, 512], F32, name="tl", tag="ps")
                    for cc in range(2):
                        nc.tensor.matmul(
                            kc_ps[:, 0:CH],
                            a[:, cc * 256 + c * 128:cc * 256 + c * 128 + 128],
                            dstb[:, si, cc, :], start=(cc == 0), stop=(cc == 1))
                    if zb:
                        nc.vector.tensor_copy(kv_sb[:, c * 128:(c + 1) * 128],
                                              kc_ps[:, 0:CH])
                    else:
                        nc.scalar.activation(kv_sb[:, c * 128:(c + 1) * 128],
                                             kc_ps[:, 0:CH], AF.Identity,
                                             bias=actx["bk"][br][:, c:c + 1])
                vt_ps = psp.tile([128, 512], F32, name="tl", tag="ps")
                for cc in range(2):
                    nc.tensor.matmul(vt_ps[:, 0:256], dstb[:, si, cc, :],
                                     a[:, 512 + cc * 256:512 + (cc + 1) * 256],
                                     start=(cc == 0), stop=(cc == 1))
                if zb:
                    nc.vector.tensor_copy(kv_sb[:, 256:512], vt_ps[:, 0:256])
                else:
                    nc.vector.tensor_add(kv_sb[:, 256:512], vt_ps[:, 0:256],
                                         actx["bv"][br][:])
                nc.sync.dma_start(agin_t[li][g][k], kv_sb[:])
                actx["left"][g] -= 1
                if actx["left"][g] == 0:
                    nc.gpsimd.collective_compute(
                        "AllGather", mybir.AluOpType.bypass, replica_groups=rg,
                        ins=[agin_t[li][g].ap().opt()],
                        outs=[agout_t[li][g].ap().opt()])

        cur_actx = new_actx(seq[0]) if seq else None
        emit_A_for(cur_actx)

        for pos, i in enumerate(seq):
            props = LAYER_PROPS[i]
            lns = LAYER_LNS[i]
            g0, g1 = grp_split(props)
            gidx = {}
            for g, idxs in enumerate([g0, g1]):
                for k, j in enumerate(idxs):
                    gidx[j] = (g, k)
            brs = sorted(set(p[0] for p in props))

            # ---- prefetch this layer's B weights
            bpk = {}
            for br in brs:
                t = wbp.tile([128, 4096], BF16, name="tl", tag="bpk")
                nc.sync.dma_start(t[:], wpackB[i, br])
                bpk[br] = t
            if not zb:
                bq_t = {br: load_bias(pbq[i, br], 256, "bq") for br in brs}
                mb_t = {br: load_bias(mbb[i, br], 256, "mb") for br in brs}
                b1_t = {br: load_bias(b1b[i, br], 512, "b1") for br in brs}
                b2_t = {br: load_bias(b2b[i, br], 256, "b2") for br in brs}

            # next layer's stage-A: prefetch + pairs whose source is already final
            nactx = new_actx(seq[pos + 1]) if pos + 1 < len(seq) else None
            if nactx is not None:
                for t in range(4):
                    if t not in lns:
                        emit_A_for(nactx, t)

            # ---- stage B: per-prop attention + merge + MLP on local queries
            ndelta = {t: sum(1 for p in props if p[1] == t) for t in range(4)}
            seen = {t: 0 for t in range(4)}
            first_delta = {t: True for t in range(4)}
            ln_done = set()

            def emit_ln(t):
                # residual + LayerNorm over channel dim (partitions x 2 ctiles)
                if "C" not in stages:
                    return
                xq = lnp.tile([128, 1024], F32, name="tl", tag="xq")
                s_ps = psp.tile([128, 512], F32, name="tl", tag="ps")
                for c in range(2):
                    nc.vector.tensor_add(xq[:, c * 256:c * 256 + CH],
                                         dstb[:, t, c, :],
                                         dlt[:, t, c * CH:(c + 1) * CH])
                    nc.vector.tensor_mul(xq[:, c * 256 + CH:c * 256 + 2 * CH],
                                         xq[:, c * 256:c * 256 + CH],
                                         xq[:, c * 256:c * 256 + CH])
                    nc.tensor.matmul(s_ps[0:64, 0:256], ones64[:],
                                     xq[:, c * 256:(c + 1) * 256],
                                     start=(c == 0), stop=(c == 1))
                st = sp_.tile([1, 512], F32, name="tl", tag="st")
                nc.vector.tensor_scalar_mul(st[:, 0:CH], s_ps[0:1, 0:CH], 1.0 / 256)
                nc.vector.tensor_scalar_mul(st[:, CH:256], s_ps[0:1, CH:256], 1.0 / 256)
                nc.vector.tensor_mul(st[:, 256:384], st[:, 0:CH], st[:, 0:CH])
                nc.vector.tensor_sub(st[:, 256:384], st[:, CH:256], st[:, 256:384])
                # rs = (var+eps)^-0.5 via ln/exp (stays in the exp table set)
                nc.scalar.activation(st[:, 384:512], st[:, 256:384], AF.Ln,
                                     bias=eps_c[:])
                nc.scalar.activation(st[:, 256:384], st[:, 384:512], AF.Exp,
                                     scale=-0.5)
                bc_ps = psp.tile([128, 512], F32, name="tl", tag="ps")
                nc.tensor.matmul(bc_ps[:, 0:CH], ones_r[:], st[:, 0:CH],
                                 start=True, stop=True)
                nc.tensor.matmul(bc_ps[:, CH:256], ones_r[:], st[:, 256:384],
                                 start=True, stop=True)
                if not ln_triv:
                    g_row = sp_.tile([1, 256], F32, name="tl", tag="grow")
                    nc.sync.dma_start(g_row[:], lng[i, t][None, :])
                    b_col = bp.tile([128, 2], F32, name="tl", tag="lnb")
                    nc.sync.dma_start(b_col[:], lnb[i, t].rearrange("(a p) -> p a", p=128))
                bc_sb = abp.tile([128, 256], F32, name="tl", tag="bcln")
                nc.vector.tensor_copy(bc_sb[:], bc_ps[:, 0:256])
                for c in range(2):
                    tmp = tmpp.tile([128, CH], F32, name="tl", tag="tmp")
                    nc.vector.tensor_sub(tmp[:], xq[:, c * 256:c * 256 + CH],
                                         bc_sb[:, 0:CH])
                    if ln_triv:
                        nc.vector.tensor_mul(dstb[:, t, c, :], tmp[:],
                                             bc_sb[:, CH:256])
                    else:
                        g_ps = psp.tile([128, 512], F32, name="tl", tag="ps")
                        nc.tensor.matmul(g_ps[:, 0:CH],
                                         g_row[:, c * 128:(c + 1) * 128],
                                         st[:, 256:384], start=True, stop=True)
                        gs = tmpp.tile([128, CH], F32, name="tl", tag="gs")
                        nc.vector.tensor_copy(gs[:], g_ps[:, 0:CH])
                        nc.vector.tensor_mul(tmp[:], tmp[:], gs[:])
                        nc.vector.tensor_scalar_add(dstb[:, t, c, :], tmp[:],
                                                    b_col[:, c:c + 1])
                # next layer's k/v for pairs sourcing t can go out now
                emit_A_for(nactx, t)

            # ---- software-pipelined prop phases:
            #   ph0: kvg gather prefetch   ph1: q + scores + exp + Z-tree
            #   ph2: A@V + Z partition-reduce + 1/Z   ph3: normalize + merge + MLP
            pstate = {}

            def ph0(j):
                g, k = gidx[j]
                kvg = kvgp.tile([128, NC * 512], BF16, name="tl", tag="kvg")
                nc.sync.dma_start(
                    kvg[:].rearrange("p (r f) -> p r f", r=NC),
                    agout_t[i][g].ap().rearrange("r q p f -> q p r f")[k])
                pstate[j] = {"kvg": kvg}

            def ph1(j):
                br, xi, si = props[j]
                b = bpk[br]
                st_ = pstate[j]
                kvg = st_["kvg"]
                if SC_ROWSPLIT:
                    qh = [qhp.tile([128, CH], BF16, name="tl", tag="qh")
                          for _ in range(2)]
                else:
                    qh = qpads[qp_ctr[0] % 2]
                    qp_ctr[0] += 1
                for c in range(2):
                    q_ps = psp.tile([128, 512], F32, name="tl", tag="ps")
                    for cc in range(2):
                        nc.tensor.matmul(
                            q_ps[:, 0:CH],
                            b[:, cc * 256 + c * 128:cc * 256 + c * 128 + 128],
                            dstb[:, xi, cc, :], start=(cc == 0), stop=(cc == 1))
                    if not SC_ROWSPLIT:
                        nc.scalar.activation(qh[c][0:64, 0:CH], q_ps[0:64, 0:CH],
                                             AF.Copy)
                        nc.scalar.activation(qh[c][64:128, CH:2 * CH],
                                             q_ps[64:128, 0:CH], AF.Copy)
                    elif zb:
                        nc.scalar.activation(qh[c][:], q_ps[:, 0:CH], AF.Copy)
                    else:
                        nc.scalar.activation(qh[c][:], q_ps[:, 0:CH], AF.Identity,
                                             bias=bq_t[br][:, c:c + 1])
                if blvl < 2:
                    st_["e_t"] = None
                    return
                e_t = []
                for mp in range(NC // 2):
                    sc_ps = scp.tile([128, 1024], F32, name="tl", tag="sc")
                    for ml in range(2):
                        m = 2 * mp + ml
                        for c in range(2):
                            if not SC_ROWSPLIT:
                                nc.tensor.matmul(
                                    sc_ps[:, ml * 512 + (2 * c) * CH:
                                          ml * 512 + (2 * c + 2) * CH],
                                    kvg[:, m * 512 + c * 128:m * 512 + (c + 1) * 128],
                                    qh[c][:], start=True, stop=True)
                            else:
                                for hh in range(2):
                                    nc.tensor.matmul(
                                        sc_ps[:, ml * 512 + (2 * c + hh) * CH:
                                              ml * 512 + (2 * c + hh + 1) * CH],
                                        kvg[64 * hh:64 * hh + 64,
                                            m * 512 + c * 128:m * 512 + (c + 1) * 128],
                                        qh[c][64 * hh:64 * hh + 64, :],
                                        start=True, stop=True,
                                        tile_position=(64 * hh, 0))
                    e_sb = ep.tile([128, 1024], BF16, name="tl", tag="exps")
                    nc.scalar.activation(e_sb[:], sc_ps[:], AF.Exp)
                    e_t.append(e_sb)
                st_["e_t"] = e_t
                if blvl >= 3:
                    t01 = ztp.tile([128, 1024], BF16, name="tl", tag="zt")
                    t23 = ztp.tile([128, 1024], BF16, name="tl", tag="zt")
                    nc.vector.tensor_add(t01[:], e_t[0][:], e_t[1][:])
                    nc.vector.tensor_add(t23[:], e_t[2][:], e_t[3][:])
                    nc.vector.tensor_add(t01[:], t01[:], t23[:])
                    z_acc = zap.tile([128, 512], BF16, name="tl", tag="zacc")
                    nc.vector.tensor_add(z_acc[:], t01[:, 0:512], t01[:, 512:1024])
                    st_["z"] = z_acc

            def ph2(j):
                st_ = pstate[j]
                kvg, e_t = st_["kvg"], st_["e_t"]
                if e_t is None:
                    return
                av_ps = [avp.tile([128, CH], F32, name="tl", tag=f"av{c}")
                         for c in range(2)]
                st_["av"] = av_ps
                for m in (range(NC) if blvl >= 4 else []):
                    e_sl = e_t[m // 2]
                    off = (m % 2) * 512
                    for h in range(H):
                        c, o = h // 2, 64 * (h % 2)
                        nc.tensor.matmul(
                            av_ps[c][o:o + 64, :],
                            kvg[:, m * 512 + 256 + 64 * h:m * 512 + 256 + 64 * h + 64],
                            e_sl[:, off + h * CH:off + (h + 1) * CH],
                            start=(m == 0), stop=(m == NC - 1),
                            tile_position=(0, o), skip_group_check=True)
                if blvl >= 5:
                    z_ps = psp.tile([128, 512], F32, name="tl", tag="ps")
                    nc.tensor.matmul(z_ps[0:64, :], ones64b[:], st_["z"][:],
                                     start=True, stop=True)
                    r_row = sp_.tile([1, 512], F32, name="tl", tag="rz")
                    nc.vector.reciprocal(r_row[:], z_ps[0:1, :])
                    st_["r"] = r_row

            def ph3(j):
                st_ = pstate[j]
                if st_["e_t"] is None or blvl < 5:
                    return
                br, xi, si = props[j]
                b = bpk[br]
                av_ps, r_row = st_["av"], st_["r"]
                bc_ps = psp.tile([128, 512], F32, name="tl", tag="ps")
                for h in range(H):
                    c, o = h // 2, 64 * (h % 2)
                    nc.tensor.matmul(bc_ps[o:o + 64, c * CH:(c + 1) * CH],
                                     ones_r[:, 0:64], r_row[:, h * CH:(h + 1) * CH],
                                     start=True, stop=True, tile_position=(0, o))
                bc_sb = abp.tile([128, 256], F32, name="tl", tag="bcav")
                nc.scalar.activation(bc_sb[:], bc_ps[:, 0:256], AF.Copy)
                attn_sb = atp.tile([128, 256], BF16, name="tl", tag="attn")
                for c in range(2):
                    nc.vector.tensor_mul(attn_sb[:, c * CH:(c + 1) * CH],
                                         av_ps[c][:], bc_sb[:, c * CH:(c + 1) * CH])
                if blvl < 6:
                    return
                m_ps = psp.tile([128, 512], F32, name="tl", tag="ps")
                for c in range(2):
                    for cc in range(2):
                        nc.tensor.matmul(
                            m_ps[:, c * CH:(c + 1) * CH],
                            b[:, 512 + cc * 256 + c * 128:512 + cc * 256 + c * 128 + 128],
                            attn_sb[:, cc * CH:(cc + 1) * CH],
                            start=(cc == 0), stop=(cc == 1))
                msg_sb = msp.tile([128, 256], BF16, name="tl", tag="msg")
                if zb:
                    nc.scalar.activation(msg_sb[:], m_ps[:, 0:256], AF.Copy)
                else:
                    for c in range(2):
                        nc.scalar.activation(msg_sb[:, c * CH:(c + 1) * CH],
                                             m_ps[:, c * CH:(c + 1) * CH],
                                             AF.Identity, bias=mb_t[br][:, c:c + 1])
                st_["msg"] = msg_sb

            def ph4(j):
                br, xi, si = props[j]
                b = bpk[br]
                st_ = pstate.pop(j)
                if st_["e_t"] is None or blvl < 7:
                    return
                msg_sb = st_["msg"]

                def h_in(cc):
                    if cc < 2:
                        return dstb[:, xi, cc, :]
                    return msg_sb[:, (cc - 2) * CH:(cc - 1) * CH]
                h1_sb = []
                for pair in range(2):
                    h_ps = psp.tile([128, 512], F32, name="tl", tag="ps")
                    for cl in range(2):
                        c = pair * 2 + cl
                        for cc in range(4):
                            nc.tensor.matmul(
                                h_ps[:, cl * CH:(cl + 1) * CH],
                                b[:, 1024 + cc * 512 + c * 128:
                                  1024 + cc * 512 + c * 128 + 128],
                                h_in(cc), start=(cc == 0), stop=(cc == 3))
                    t_ = h1p.tile([128, 256], BF16, name="tl", tag="h1")
                    if zb:
                        nc.vector.tensor_relu(t_[:], h_ps[:, 0:256])
                    else:
                        for cl in range(2):
                            c = pair * 2 + cl
                            nc.scalar.activation(t_[:, cl * CH:(cl + 1) * CH],
                                                 h_ps[:, cl * CH:(cl + 1) * CH],
                                                 AF.Relu, bias=b1_t[br][:, c:c + 1])
                    h1_sb.append(t_)
                if blvl < 8:
                    return
                d_ps = psp.tile([128, 512], F32, name="tl", tag="ps")
                for c in range(2):
                    for cc in range(4):
                        nc.tensor.matmul(
                            d_ps[:, c * CH:(c + 1) * CH],
                            b[:, 3072 + cc * 256 + c * 128:
                              3072 + cc * 256 + c * 128 + 128],
                            h1_sb[cc // 2][:, (cc % 2) * CH:(cc % 2 + 1) * CH],
                            start=(cc == 0), stop=(cc == 3))
                if zb:
                    if first_delta[xi]:
                        nc.vector.tensor_copy(dlt[:, xi, :], d_ps[:, 0:256])
                    else:
                        nc.vector.tensor_add(dlt[:, xi, :], dlt[:, xi, :],
                                             d_ps[:, 0:256])
                else:
                    tmp = tmpp.tile([128, 256], F32, name="tl", tag="dtmp")
                    for c in range(2):
                        nc.scalar.activation(tmp[:, c * CH:(c + 1) * CH],
                                             d_ps[:, c * CH:(c + 1) * CH],
                                             AF.Identity, bias=b2_t[br][:, c:c + 1])
                    if first_delta[xi]:
                        nc.vector.tensor_copy(dlt[:, xi, :], tmp[:])
                    else:
                        nc.vector.tensor_add(dlt[:, xi, :], dlt[:, xi, :], tmp[:])
                first_delta[xi] = False
                seen[xi] += 1
                if seen[xi] == ndelta[xi] and xi in lns and xi not in ln_done:
                    ln_done.add(xi)
                    emit_ln(xi)

            pl = (g0 + g1) if "B" in stages else []
            n = len(pl)
            if n:
                ph0(pl[0])
            for idx in range(n + 3):
                if idx + 1 < n:
                    ph0(pl[idx + 1])
                if 0 <= idx - 3 < n:
                    ph4(pl[idx - 3])
                if 0 <= idx - 2 < n:
                    ph3(pl[idx - 2])
                if 0 <= idx - 1 < n:
                    ph2(pl[idx - 1])
                if idx < n:
                    ph1(pl[idx])

            if "C" in stages and "B" not in stages:
                for t in lns:
                    emit_ln(t)

        # ---- epilogue: out[m] = (1/32) qvec^T kmat[:, m]
        s1 = sp_.tile([128, 2], F32, name="tl", tag="s1")
        for c in range(2):
            nc.vector.reduce_sum(s1[:, c:c + 1], dstb[:, 1, c, :],
                                 axis=mybir.AxisListType.X)
            nc.sync.dma_start(ag2in[c], s1[:, c:c + 1])
        nc.gpsimd.collective_compute(
            "AllGather", mybir.AluOpType.bypass, replica_groups=rg,
            ins=[ag2in.ap().opt()], outs=[ag2out.ap().opt()])
        gath = sp_.tile([128, NC], F32, name="tl", tag="gath")
        d1b = sp_.tile([128, 2], F32, name="tl", tag="d1b")
        d1bb = sp_.tile([128, 2], BF16, name="tl", tag="d1bb")
        for c in range(2):
            nc.sync.dma_start(gath[:], ag2out.ap().rearrange("r c p o -> c p (r o)")[c])
            nc.vector.reduce_sum(d1b[:, c:c + 1], gath[:], axis=mybir.AxisListType.X)
        nc.vector.tensor_copy(d1bb[:], d1b[:])

        wq5 = [cpool.tile([128, 256], BF16, name="tl", tag=f"wq5{k}") for k in range(2)]
        wk5 = [cpool.tile([128, 256], BF16, name="tl", tag=f"wk5{k}") for k in range(2)]
        for k in range(2):
            nc.sync.dma_start(wq5[k][:], w5T[0, k * 128:(k + 1) * 128, :])
            nc.sync.dma_start(wk5[k][:], w5T[1, k * 128:(k + 1) * 128, :])
        b5 = bp.tile([128, 4], F32, name="tl", tag="b5")
        nc.sync.dma_start(b5[:], pb5.rearrange("t (a p) -> p (t a)", p=128))
        qv = sp_.tile([128, 2], F32, name="tl", tag="qv")
        for c in range(2):
            q_ps = psp.tile([128, 512], F32, name="tl", tag="ps")
            for cc in range(2):
                nc.tensor.matmul(q_ps[:, 0:1], wq5[cc][:, c * 128:(c + 1) * 128],
                                 d1bb[:, cc:cc + 1], start=(cc == 0), stop=(cc == 1))
            nc.scalar.activation(qv[:, c:c + 1], q_ps[:, 0:1], AF.Identity,
                                 bias=b5[:, c:c + 1], scale=1.0 / N)
        km = [sp_.tile([128, CH], F32, name="tl", tag=f"km{c}") for c in range(2)]
        for c in range(2):
            k_ps = psp.tile([128, 512], F32, name="tl", tag="ps")
            for cc in range(2):
                nc.tensor.matmul(k_ps[:, 0:CH], wk5[cc][:, c * 128:(c + 1) * 128],
                                 dstb[:, 0, cc, :], start=(cc == 0), stop=(cc == 1))
            nc.scalar.activation(km[c][:], k_ps[:, 0:CH], AF.Identity,
                                 bias=b5[:, 2 + c:3 + c])
        o_ps = psp.tile([128, 512], F32, name="tl", tag="ps")
        for c in range(2):
            nc.vector.tensor_scalar_mul(km[c][:], km[c][:], qv[:, c:c + 1])
            nc.tensor.matmul(o_ps[0:64, 0:CH], ones64[:], km[c][:],
                             start=(c == 0), stop=(c == 1))
        o_sb = sp_.tile([1, CH], F32, name="tl", tag="osb")
        nc.scalar.activation(o_sb[:], o_ps[0:1, 0:CH], AF.Copy, scale=1.0 / 32)
        nc.sync.dma_start(out_d[:], o_sb[:])

    nc.compile()
    return nc


def prep_inputs(inputs, zb=True, ln_triv=True):
    inp = {k: np.ascontiguousarray(np.asarray(v)) for k, v in inputs.items()}
    pw, pb = inp['proj_w'].astype(np.float32), inp['proj_b'].astype(np.float32)
    mw = inp['merge_w'].astype(np.float32)
    w1 = inp['mlp_w1'].astype(np.float32)
    w2 = inp['mlp_w2'].astype(np.float32)

    bf = mybir.dt.np(mybir.dt.bfloat16)
    wpackA = np.empty((5, 3, 128, 1024), np.float32)
    wpackB = np.empty((5, 3, 128, 4096), np.float32)
    pbq = np.empty((5, 3, 256), np.float32)
    pbk = np.empty((5, 3, 256), np.float32)
    pbv = np.empty((5, 3, 256), np.float32)
    for i in range(5):
        for br in range(3):
            wqT = pw[br, i, 0][PERM].T * 0.125   # [256 in, 256 out]
            wkT = pw[br, i, 1][PERM].T
            wvT = pw[br, i, 2][PERM].T
            mgT = mw[br, i][:, PERM].T
            w1T = w1[br, i].T                     # [512, 512]
            w2T = w2[br, i].T                     # [512, 256]
            wpackA[i, br, :, 0:256] = wkT[0:128]
            wpackA[i, br, :, 256:512] = wkT[128:256]
            wpackA[i, br, :, 512:768] = wvT[0:128]
            wpackA[i, br, :, 768:1024] = wvT[128:256]
            wpackB[i, br, :, 0:256] = wqT[0:128]
            wpackB[i, br, :, 256:512] = wqT[128:256]
            wpackB[i, br, :, 512:768] = mgT[0:128]
            wpackB[i, br, :, 768:1024] = mgT[128:256]
            for cc in range(4):
                wpackB[i, br, :, 1024 + cc * 512:1024 + (cc + 1) * 512] = \
                    w1T[cc * 128:(cc + 1) * 128]
                wpackB[i, br, :, 3072 + cc * 256:3072 + (cc + 1) * 256] = \
                    w2T[cc * 128:(cc + 1) * 128]
            pbq[i, br] = pb[br, i, 0][PERM] * 0.125
            pbk[i, br] = pb[br, i, 1][PERM]
            pbv[i, br] = pb[br, i, 2][PERM]

    w5T = np.stack([pw[0, 5, 0].T, pw[0, 5, 1].T]).astype(bf)
    pb5 = np.stack([pb[0, 5, 0], pb[0, 5, 1]]).astype(np.float32)

    desc = np.stack([inp[f'desc{t}'][0] for t in range(4)]).astype(np.float32)
    shared = dict(wpackA=wpackA.astype(bf), wpackB=wpackB.astype(bf),
                  w5T=w5T, pb5=pb5)
    if not zb:
        mb = inp['merge_b'].astype(np.float32)
        b1 = inp['mlp_b1'].astype(np.float32)
        b2 = inp['mlp_b2'].astype(np.float32)
        shared.update(
            pbq=pbq, pbk=pbk, pbv=pbv,
            mbb=np.ascontiguousarray(np.transpose(mb[:, :5], (1, 0, 2))),
            b1b=np.ascontiguousarray(np.transpose(b1[:, :5], (1, 0, 2))),
            b2b=np.ascontiguousarray(np.transpose(b2[:, :5], (1, 0, 2))))
    if not ln_triv:
        ng = inp['norm_g'].astype(np.float32)
        nb = inp['norm_b'].astype(np.float32)
        shared.update(
            lng=np.ascontiguousarray(np.transpose(ng[:, :5], (1, 0, 2))),
            lnb=np.ascontiguousarray(np.transpose(nb[:, :5], (1, 0, 2))))

    in_maps = []
    for j in range(NC):
        xcj = desc[:, :, j * CH:(j + 1) * CH].reshape(4, 2, 128, CH)
        xcb = np.ascontiguousarray(np.transpose(xcj, (2, 0, 1, 3))).astype(bf)
        in_maps.append({"xcb": xcb, **shared})
    return in_maps


def kernel(**inputs):
    zb = all(not np.asarray(inputs[k]).any() for k in
             ("proj_b", "merge_b", "mlp_b1", "mlp_b2"))
    ln_triv = (np.asarray(inputs["norm_g"])[:, :5] == 1).all() and \
        not np.asarray(inputs["norm_b"])[:, :5].any()
    key = f"nc{zb}_{ln_triv}"
    if key not in _cache:
        _cache[key] = build_kernel(zb=zb, ln_triv=ln_triv)
    nc = _cache[key]
    in_maps = prep_inputs(inputs, zb=zb, ln_triv=ln_triv)
    res = run_bass_kernel_spmd(nc, in_maps, core_ids=list(range(NC)))
    out = np.concatenate([res.results[j]["out"][0] for j in range(NC)])
    mask = np.asarray(inputs["unreachable"]).any(axis=0)
    out = np.where(mask, np.float32(-1e9), out.astype(np.float32))
    return out
